# revision 2
# baseline (speedup 1.0000x reference)
"""BiLSTM-CRF kernel for Trainium2 (8 NeuronCores, SPMD batch-sharded).

Device (Bass/Tile, one launch per call, 8 cores x 4 sequences):
  phase 1: xg = [x|1] @ [Wih.T;b] for both directions (PE, fp32r)
  phase 2: both LSTM recurrences, position-indexed For_i hardware loop
           (fwd reads/writes col t, bwd col L-1-t; no data reversal)
  phase 3: emissions^T = W_out @ hcat (PE), DMA'd out (32KB/core)
Host: embedding gather (shard prep), Viterbi decode (tiny: T=4).

Transfers per call: ~8.3MB/core up, 32KB/core down -- vs 300+MB for the
gates-on-host split. The axon device init (~80s) is paid at import time.
"""

import sys
import time

for _p in ("/opt/trn_rl_repo", "/root/.axon_site/_ro/trn_rl_repo"):
    if _p not in sys.path:
        sys.path.insert(0, _p)

import numpy as np

B, L, V, E, H, T = 32, 512, 100000, 300, 256, 4
NCORES = 8
S = B // NCORES          # sequences per core
E1 = 384                 # E rows + bias row + pad to 3*128 (SBUF layout)
E2 = 301                 # rows actually uploaded (E + bias row)
KE = E1 // 128           # 3 contraction blocks for the input projection
G4 = 4 * H               # 1024 gates per direction
MB = G4 // 128           # 8 gate M-blocks per direction
KH = H // 128            # 2 contraction blocks for the recurrence
KC = 2 * H // 128        # 4 contraction blocks for the emission projection

LAST_DEVICE_NS = None    # wall-time of the device execution, for test.py
_NC_CACHE = {}


def _bilstm_ir(tc, L_steps, xT, wihT, wihTr, whhT, woutT, emisT):
    """Emit the full BiLSTM IR. All APs are DRAM tensors:
    xT    [E1, S*L]        tokens col = s*L + t; row 300 = 1.0 (bias), pad 0
    wihT  [E1, 2*G4]       cols 0:G4 fwd gates, G4:2*G4 bwd; gate order i,f,o,g
    whhT  [128, 4*G4]      (k,m): k in {f0,f1,b0,b1} blocks of h; m gate dim
    woutT [128, KC*T]      (k,t): k blocks of hcat = [h_f | h_b]
    emisT [T, S*L]         emissions (pre b_out), col = s*L + t
    """
    import concourse.bass as bass
    import concourse.mybir as mybir
    from concourse.bass import ds

    from contextlib import ExitStack

    nc = tc.nc
    NT = S * L_steps
    f32 = mybir.dt.float32
    f16 = mybir.dt.float16
    dt_p1 = f16   # xT, wihT (phase-1 matmul operands)
    dt_rec = f32  # whh, hseq, wout (recurrence + emission operands)
    ACT = mybir.ActivationFunctionType

    ctx = ExitStack()
    pool = ctx.enter_context(tc.tile_pool(name="main", bufs=1))

    # --- load inputs to SBUF (E-dim tensors ship 301 rows; tail zeroed) ---
    xT_sb = pool.tile([128, KE, NT], dt_p1, tag="slotA")
    wihT_sb = pool.tile([128, KE, 2 * G4], dt_p1, tag="slotB")
    wihTr_sb = pool.tile([128, KE, 2 * G4], dt_p1, tag="wihr")
    whh_sb = pool.tile([128, 2 * KH, G4], dt_rec, tag="whh")
    wout_sb = pool.tile([128, KC, T], dt_rec, tag="wout")
    PT = E2 - 256  # partial-block rows (45)
    for sb, dram in ((xT_sb, xT), (wihT_sb, wihT), (wihTr_sb, wihTr)):
        nc.vector.memset(sb[:, KE - 1, :], 0.0)  # zero last k-block; DMA then fills rows 0:45
        nc.sync.dma_start(
            sb[:, : KE - 1, :], dram[:256].rearrange("(k p) n -> p k n", p=128)
        )
        nc.sync.dma_start(sb[:PT, KE - 1, :], dram[256:E2])
    nc.sync.dma_start(whh_sb[:], whhT.rearrange("p (k m) -> p k m", m=G4))
    nc.sync.dma_start(wout_sb[:], woutT.rearrange("p (k t) -> p k t", t=T))

    xg = [
        pool.tile([128, MB, NT], f32, tag="xg_f", name="xg_f"),
        pool.tile([128, MB, NT], f32, tag="xg_b", name="xg_b"),
    ]

    # --- phase 1: input projections, out = wihT.T @ xT (per 128-col M block) ---
    NCHUNK = 512
    with tc.tile_pool(name="ps1", bufs=4, space="PSUM") as ps1:
        for d in range(2):              # direction
            for m in range(MB):         # gate M-block
                for c0 in range(0, NT, NCHUNK):
                    cw = min(NCHUNK, NT - c0)
                    pt = ps1.tile([128, NCHUNK], f32, tag="p1", name="p1")
                    for w_i, w_sb in enumerate((wihT_sb, wihTr_sb)):
                        for k in range(KE):
                            nc.tensor.matmul(
                                pt[:, :cw],
                                w_sb[:, k, d * G4 + m * 128 : d * G4 + (m + 1) * 128],
                                xT_sb[:, k, c0 : c0 + cw],
                                start=(w_i == 0 and k == 0),
                                stop=(w_i == 1 and k == KE - 1),
                            )
                    nc.vector.tensor_copy(out=xg[d][:, m, c0 : c0 + cw], in_=pt[:, :cw])

    # --- phase 2: the two recurrences ---
    # hseq layout [128, KH, NT]; gate/act working layout [128, m, s].
    hseq = [
        pool.tile([128, KH, NT], dt_rec, tag="slotA", name="hseq_f"),
        pool.tile([128, KH, NT], dt_rec, tag="slotB", name="hseq_b"),
    ]
    acts = [pool.tile([128, MB, S], f32, tag=f"acts{d}", name=f"acts{d}") for d in range(2)]
    gsum = [pool.tile([128, MB, S], f32, tag=f"gsum{d}", name=f"gsum{d}") for d in range(2)]
    cc = [pool.tile([128, KH, S], f32, tag=f"c{d}", name=f"c{d}") for d in range(2)]
    tmp = [pool.tile([128, KH, S], f32, tag=f"tmp{d}", name=f"tmp{d}") for d in range(2)]
    tch = [pool.tile([128, KH, S], f32, tag=f"tch{d}", name=f"tch{d}") for d in range(2)]

    xg_r = [t.rearrange("p m (s t) -> p m s t", s=S) for t in xg]
    hseq_r = [t.rearrange("p k (s t) -> p k s t", s=S) for t in hseq]

    def lstm_tail(d, gate_src, col_w):
        """gate_src [128, MB, S] view of pre-activations; writes c and hseq."""
        a = acts[d]
        nc.scalar.activation(a[:, 0:6, :], gate_src[:, 0:6, :], ACT.Sigmoid)
        nc.scalar.activation(a[:, 6:8, :], gate_src[:, 6:8, :], ACT.Tanh)
        return a

    def lstm_step0(d, col):
        a = lstm_tail(d, xg_r[d][:, :, :, col], col)
        # c0 = sig(i) * tanh(g);  h0 = sig(o) * tanh(c0)
        nc.vector.tensor_mul(out=cc[d][:], in0=a[:, 0:2, :], in1=a[:, 6:8, :])
        nc.scalar.activation(tch[d][:], cc[d][:], ACT.Tanh)
        nc.vector.tensor_mul(
            out=hseq_r[d][:, :, :, col], in0=a[:, 4:6, :], in1=tch[d][:]
        )

    def lstm_step(ps2, d, col_r, col_g, col_w):
        pt = ps2.tile([128, MB, S], f32, tag=f"p2_{d}", name=f"p2_{d}")
        for m in range(MB):
            for k in range(KH):
                nc.tensor.matmul(
                    pt[:, m, :],
                    whh_sb[:, d * KH + k, m * 128 : (m + 1) * 128],
                    hseq_r[d][:, k, :, col_r],
                    start=(k == 0),
                    stop=(k == KH - 1),
                )
        nc.vector.tensor_add(out=gsum[d][:], in0=pt[:], in1=xg_r[d][:, :, :, col_g])
        a = lstm_tail(d, gsum[d], col_w)
        # c = sig(f)*c + sig(i)*tanh(g);  h = sig(o)*tanh(c)
        nc.vector.tensor_mul(out=tmp[d][:], in0=a[:, 0:2, :], in1=a[:, 6:8, :])
        nc.vector.tensor_mul(out=cc[d][:], in0=a[:, 2:4, :], in1=cc[d][:])
        nc.vector.tensor_add(out=cc[d][:], in0=cc[d][:], in1=tmp[d][:])
        nc.scalar.activation(tch[d][:], cc[d][:], ACT.Tanh)
        nc.vector.tensor_mul(
            out=hseq_r[d][:, :, :, col_w], in0=a[:, 4:6, :], in1=tch[d][:]
        )

    lstm_step0(0, 0)
    lstm_step0(1, L_steps - 1)
    with tc.tile_pool(name="ps2", bufs=2, space="PSUM") as ps2:
        with tc.For_i(0, L_steps - 1, 1) as i:
            lstm_step(ps2, 0, ds(i, 1), ds(i + 1, 1), ds(i + 1, 1))
            lstm_step(
                ps2,
                1,
                ds(L_steps - 1 - i, 1),
                ds(L_steps - 2 - i, 1),
                ds(L_steps - 2 - i, 1),
            )

    # --- phase 3: emissions^T = woutT.T @ hcatT ---
    emis_sb = pool.tile([T, NT], f32, tag="emis")
    with tc.tile_pool(name="ps3", bufs=2, space="PSUM") as ps3:
        for c0 in range(0, NT, NCHUNK):
            cw = min(NCHUNK, NT - c0)
            pt = ps3.tile([T, NCHUNK], f32, tag="p3", name="p3")
            for k in range(KC):
                nc.tensor.matmul(
                    pt[:, :cw],
                    wout_sb[:, k, :],
                    hseq[k // KH][:, k % KH, c0 : c0 + cw],
                    start=(k == 0),
                    stop=(k == KC - 1),
                )
            nc.vector.tensor_copy(out=emis_sb[:, c0 : c0 + cw], in_=pt[:, :cw])
    nc.sync.dma_start(emisT, emis_sb[:])
    ctx.close()


def build_nc(L_steps=L):
    import concourse.bacc as bacc
    import concourse.mybir as mybir
    from concourse.tile import TileContext

    NT = S * L_steps
    f32 = mybir.dt.float32
    f16 = mybir.dt.float16
    nc = bacc.Bacc()
    xT = nc.declare_dram_parameter("xT", [E2, NT], f16, isOutput=False)
    wihT = nc.declare_dram_parameter("wihT", [E2, 2 * G4], f16, isOutput=False)
    wihTr = nc.declare_dram_parameter("wihTr", [E2, 2 * G4], f16, isOutput=False)
    whhT = nc.declare_dram_parameter("whhT", [128, 2 * KH * G4], f32, isOutput=False)
    woutT = nc.declare_dram_parameter("woutT", [128, KC * T], f32, isOutput=False)
    emisT = nc.declare_dram_parameter("emisT", [T, NT], f32, isOutput=True)
    with TileContext(nc) as tc:
        _bilstm_ir(tc, L_steps, xT[:], wihT[:], wihTr[:], whhT[:], woutT[:], emisT[:])
    nc.finalize()
    return nc


_PERM = None


def _gate_perm():
    """Row permutation taking PyTorch gate order (i,f,g,o) to (i,f,o,g)."""
    global _PERM
    if _PERM is None:
        r = np.arange(G4)
        _PERM = np.concatenate([r[0:256], r[256:512], r[768:1024], r[512:768]])
    return _PERM


def host_inputs(x, Wih_f, b_f, Wih_b, b_b, Whh_f, Whh_b, W_out, L_steps=L):
    """Build per-core input maps. x: [B, L, E] fp32 (B divisible by NCORES)."""
    p = _gate_perm()
    NT = S * L_steps

    wih32 = np.zeros((E2, 2 * G4), np.float32)
    wih32[:E, 0:G4] = Wih_f[p].T
    wih32[E, 0:G4] = b_f[p]
    wih32[:E, G4:] = Wih_b[p].T
    wih32[E, G4:] = b_b[p]
    wihT = wih32.astype(np.float16)
    wihTr = (wih32 - wihT.astype(np.float32)).astype(np.float16)

    def whh_pack(Whh):
        # [128, KH, G4]: (part, k) = h-dim, m = gate dim (reordered)
        return np.ascontiguousarray(
            Whh[p].T.reshape(KH, 128, G4).transpose(1, 0, 2)
        ).reshape(128, KH * G4)

    whhT = np.concatenate([whh_pack(Whh_f), whh_pack(Whh_b)], axis=1)
    woutT = np.ascontiguousarray(
        W_out.T.reshape(KC, 128, T).transpose(1, 0, 2)
    ).reshape(128, KC * T)

    in_maps = []
    for c in range(NCORES):
        xc = x[c * S : (c + 1) * S].reshape(NT, E)
        xTp = np.empty((E2, NT), np.float16)
        xTp[:E] = xc.T
        xTp[E] = 1.0
        in_maps.append(
            {"xT": xTp, "wihT": wihT, "wihTr": wihTr, "whhT": whhT, "woutT": woutT}
        )
    return in_maps


def _viterbi(emissions, mask, transitions, start_trans, end_trans):
    Bn, Ln, _ = emissions.shape
    m = mask.astype(bool)
    score = start_trans + emissions[:, 0]
    history = np.empty((Ln - 1, Bn, T), np.int32)
    for t in range(1, Ln):
        cand = score[:, :, None] + transitions[None] + emissions[:, t][:, None, :]
        history[t - 1] = np.argmax(cand, axis=1).astype(np.int32)
        new = np.max(cand, axis=1)
        score = np.where(m[:, t][:, None], new, score)
    score = score + end_trans
    tag = np.argmax(score, axis=-1).astype(np.int32)
    tags = np.empty((Bn, Ln), np.int32)
    tags[:, Ln - 1] = tag
    rows = np.arange(Bn)
    for t in range(Ln - 2, -1, -1):
        prev = history[t][rows, tag]
        tag = np.where(m[:, t + 1], prev, tag).astype(np.int32)
        tags[:, t] = tag
    return tags * mask.astype(np.int32)


def _get_nc():
    if "nc" not in _NC_CACHE:
        _NC_CACHE["nc"] = build_nc()
    return _NC_CACHE["nc"]


def _run_device(in_maps):
    global LAST_DEVICE_NS
    from concourse.bass_utils import run_bass_kernel_spmd

    nc = _get_nc()
    t0 = time.perf_counter()
    res = run_bass_kernel_spmd(nc, in_maps, list(range(NCORES)))
    LAST_DEVICE_NS = int((time.perf_counter() - t0) * 1e9)
    if getattr(res, "exec_time_ns", None):
        LAST_DEVICE_NS = int(res.exec_time_ns)
    return [np.asarray(r["emisT"]) for r in res.results]


def kernel(
    word_ids,
    mask,
    label_ids,
    emb,
    Wih_f,
    Whh_f,
    b_f,
    Wih_b,
    Whh_b,
    b_b,
    W_out,
    b_out,
    transitions,
    start_trans,
    end_trans,
):
    word_ids = np.asarray(word_ids, np.int32)
    mask = np.asarray(mask, np.int32)
    emb = np.asarray(emb, np.float32)

    x = emb[word_ids]  # [B, L, E] embedding gather (host; shard prep)

    in_maps = host_inputs(
        x,
        np.asarray(Wih_f, np.float32),
        np.asarray(b_f, np.float32),
        np.asarray(Wih_b, np.float32),
        np.asarray(b_b, np.float32),
        np.asarray(Whh_f, np.float32),
        np.asarray(Whh_b, np.float32),
        np.asarray(W_out, np.float32),
    )
    outs = _run_device(in_maps)

    # emisT [T, S*L] per core -> emissions [B, L, T]
    emissions = np.concatenate(
        [o.reshape(T, S, L).transpose(1, 2, 0) for o in outs], axis=0
    ) + np.asarray(b_out, np.float32)

    return _viterbi(
        emissions,
        mask,
        np.asarray(transitions, np.float32),
        np.asarray(start_trans, np.float32),
        np.asarray(end_trans, np.float32),
    ).astype(np.int32)


def warmup():
    """Pay axon device init + one compile at import/module-load time."""
    try:
        zero = np.zeros((B, L, E), np.float32)
        zw = np.zeros((G4, E), np.float32)
        zb = np.zeros((G4,), np.float32)
        zh = np.zeros((G4, H), np.float32)
        zo = np.zeros((T, 2 * H), np.float32)
        _run_device(host_inputs(zero, zw, zb, zw, zb, zh, zh, zo))
    except Exception:
        pass


import os as _os

if not _os.environ.get("BILSTM_KERNEL_NO_WARMUP"):
    warmup()
